# revision 36
# baseline (speedup 1.0000x reference)
"""Trainium2 Bass kernel for nn_Blurring_Model: 3D Gaussian blur (9^3 PSF)
on x[8,1,128,128,128] fp32, data-parallel over batch across 8 NeuronCores.

Method (per core, volume V[128,128,128]):
  The 3D PSF is separable: three 9-tap 1D convolutions along D, H, W.
  Each 1D conv along the SBUF partition axis is a matmul with a banded
  128x128 matrix B (B[d, d'] = g[d - d' + 4]); zero band entries handle
  the 'same' zero padding exactly.

  Every pass uses the volume chunk as the matmul's stationary operand
  (lhsT) and streams B, so out = chunk^T @ B both convolves the current
  partition axis AND rotates the next axis onto partitions ("rotation
  matmul") - no explicit transposes.

  v2 design notes (measured on HW, each vs the previous state):
  - fp16 on-chip everywhere (matmuls N=128 stream in 128 PE cycles;
    the f32r path streamed N=256 at half the rate).
  - x is pre-transposed on the HOST to [d, (w, h)] so pass-1 stationary
    chunks are contiguous 128-col blocks; the input DMA is split into
    16 chunk-blocks on one HWDGE ring (the 4MB load is HBM-bound, a
    second ring does not help) and pass 1 chases it.
  - Intermediate volumes use a d'-split layout
        V[p, (n//8)*1024 + m*8 + (n%8)]
    (m = next pass's stationary axis, n = chunk index): weight loads
    read at a single 16B stride and the PSUM evacuation copies keep
    8-element contiguous runs.  Fully-strided 2-byte weight loads
    degraded the copy engines' SBUF throughput 60-90% (sub-bank
    hammering); element-wise scatter copies were ~3.5x slow.  This
    layout keeps the PE at K=8/8 (2.4 GHz, ~56-62ns per LDW+MM pair)
    with the copies pipelined at ~1.1-1.2us per 1024 elems.
  - One PSUM tile spanning all 8 banks as 4 rotating quarters; each
    group's quarter is evacuated by one copy, alternating DVE/ACT.
  - Output volume is fp16 (host upcasts); each 1024-block is DMA'd on
    the Sync queue as soon as its copy lands (short post-kernel drain).
  - ~2us PE warmup covers the first chunk's DMA latency and starts the
    HAM un-throttle without delaying pass-1's first evacuation copies.

  Layouts:
    V0 [d, w*128 + h]        --pass1 conv D (chunk w)--> out [h, d']
    V1 [h, split(d', w)]     --pass2 conv H (chunk d')--> out [w, h']
    V2 [w, split(h', d')]    --pass3 conv W (chunk h')--> out [d', w']
    V3 [d, h*128 + w]        == final y layout, DMA'd out as fp16.
"""

import sys

if "/opt/trn_rl_repo" not in sys.path:
    sys.path.insert(0, "/opt/trn_rl_repo")

import numpy as np

KERNEL_SIZE = 9
SPACING = (4.0, 4.0, 4.0)
CENTER = (KERNEL_SIZE - 1) / 2.0
PAD = (KERNEL_SIZE - 1) // 2
P = 128
HW = P * P
N_CORES = 8

GRP = 8            # matmul chunks per PSUM group / copy
NGRP = P // GRP    # groups per pass
NB = 128           # B matrix columns (= matmul N)
# PE warmup matmuls (~2us): covers the first input chunk's DMA latency and
# starts the HAM un-throttle without delaying pass-1's first evacuation
# copies (pass 1's tail is copy-bound, so a late start costs wall time;
# swept 12/20/28/48 on HW -- 20 gave the best 3-run mean).
N_WARM = 20

_cache = {}


def _gauss1d(sigma, spacing):
    s = float(sigma) / spacing
    xs = np.arange(KERNEL_SIZE, dtype=np.float64)
    g = np.exp(-((xs - CENTER) ** 2) / (2.0 * s * s))
    g = g / g.sum()
    return g.astype(np.float32)


def _banded(g, ncols):
    # B[d, d'] = g[d - d' + PAD] for |d - d'| <= PAD, else 0.
    B = np.zeros((P, ncols), dtype=np.float32)
    d = np.arange(P)
    for i in range(KERNEL_SIZE):
        off = i - PAD
        dp = d - off
        m = (dp >= 0) & (dp < P)
        B[d[m], dp[m]] = g[i]
    return B


def _build():
    """Builds the SPMD Bass module (single program, run on 8 cores)."""
    if "v2" in _cache:
        return _cache["v2"]

    from contextlib import ExitStack

    import concourse.bacc as bacc
    import concourse.mybir as mybir
    import concourse.tile as tile
    from concourse.tile_rust import add_dep_helper

    f32 = mybir.dt.float32
    f16 = mybir.dt.float16

    nc = bacc.Bacc(trn_type="TRN2", target_bir_lowering=False, debug=False)
    x_in = nc.declare_dram_parameter("x", [P, HW], f16, isOutput=False).ap()
    b_in = nc.declare_dram_parameter("bmats", [P, 3 * NB], f16, isOutput=False).ap()
    y_out = nc.declare_dram_parameter("y", [P, HW], f16, isOutput=True).ap()

    with ExitStack() as ctx:
        tc = ctx.enter_context(tile.TileContext(nc))
        vol = ctx.enter_context(tc.tile_pool(name="vol", bufs=4))
        consts = ctx.enter_context(tc.tile_pool(name="consts", bufs=1))
        pspool = ctx.enter_context(tc.tile_pool(name="ps", bufs=1, space="PSUM"))

        braw = consts.tile([P, 3 * NB], f16, name="braw", tag="braw")
        # braw rides the Scalar HWDGE ring (one early descriptor, before any
        # copies are queued there) so the Sync ring starts streaming input
        # chunk 0 immediately at preamble end.
        nc.scalar.dma_start(out=braw[:], in_=b_in[:])
        scratch = consts.tile([P, 128], f32, name="scratch", tag="scratch")

        v0 = vol.tile([P, HW], f16, name="v0", tag="vol")
        v1 = vol.tile([P, HW], f16, name="v1", tag="vol")
        v2 = vol.tile([P, HW], f16, name="v2", tag="vol")
        v3 = vol.tile([P, HW], f16, name="v3", tag="vol")

        # Input DMA, split into 16 chunk-blocks that pass-1 groups chase,
        # all on the Sync HWDGE ring (issued after braw).  The 4MB load is
        # HBM-bandwidth-bound (~358 GB/s per core), so a second ring cannot
        # speed it up -- it only delays the chase chunks.
        for g in range(NGRP):
            nc.sync.dma_start(
                out=v0[:, g * (GRP * NB) : (g + 1) * (GRP * NB)],
                in_=x_in[:, g * (GRP * NB) : (g + 1) * (GRP * NB)],
            )

        # One persistent PSUM tile spanning all 8 banks, divided into 4
        # rotating quarters (2 banks each) indexed by group%4.  Each group's
        # quarter is evacuated by one flat 1024-elem copy, alternating
        # Vector (even groups -> quarters 0/2) and Scalar (odd -> 1/3).
        QSZ = GRP * NB
        ps_all = pspool.tile([P, 4 * QSZ], f32, name="ps_all", tag="ps")

        def ps_q(g):
            q = g % 4
            return ps_all[:, q * QSZ : (q + 1) * QSZ]

        # Warm the ACT tables (Copy) and the PE HAM clock gate while the
        # first input chunk's DMA is in flight.
        nc.scalar.copy(out=scratch[:], in_=braw[:, 0:128])
        for _ in range(N_WARM):
            nc.tensor.matmul(
                out=ps_q(3)[:, 0:NB],
                lhsT=braw[:, 0:128],
                rhs=braw[:, 0:NB],
                start=True,
                stop=True,
            )

        SPL = 16  # d'-split lane width (elements per contiguous run)

        def split_chunk(src, i):
            # d'-split source V[p, nh*SPL*128 + m*SPL + nl] (n = nh*SPL+nl):
            # chunk i -> [p, m:128] single stride SPL (legal 2-D weights AP)
            return src.rearrange("p (nh m nl) -> p nh nl m", nh=P // SPL, m=P, nl=SPL)[
                :, i // SPL, i % SPL, :
            ]

        def block_chunk(src, i):
            return src[:, i * 128 : (i + 1) * 128]

        def conv_pass(src, dst, b_idx, src_split, dst_split, pass_idx):
            # Weight loads with a 2-byte fully-strided AP degrade the copy
            # engines' SBUF throughput by 60-90% (sub-bank hammering), and
            # element-wise scatter copies are ~3.5x slow (one 16B line touch
            # per 2-byte write).  The compromise: intermediate volumes use a
            # d'-split layout  V[p, (n//8)*1024 + m*8 + (n%8)]  where m is
            # the next pass's stationary axis.  Weight loads then read at a
            # single 16B stride (spread across sub-banks) and the evacuation
            # copies keep 8-element contiguous runs (same 16B-line count as
            # a flat copy).
            b_ap = braw[:, b_idx * NB : (b_idx + 1) * NB]
            chunk_fn = split_chunk if src_split else block_chunk
            dve_copies = []
            for g in range(NGRP):
                p = ps_q(g)
                for c in range(GRP):
                    nc.tensor.matmul(
                        out=p[:, c * NB : (c + 1) * NB],
                        lhsT=chunk_fn(src, g * GRP + c),
                        rhs=b_ap,
                        start=True,
                        stop=True,
                    )
                if dst_split:
                    # PSUM [p, (c, nh, nl)] -> dst[p, nh*SPL*128 + (g*8+c)*SPL + nl]
                    src_ap = p.rearrange("p (c nh nl) -> p c nh nl", c=GRP, nh=P // SPL)
                    dst_ap = dst.rearrange(
                        "p (nh m nl) -> p m nh nl", nh=P // SPL, m=P, nl=SPL
                    )[:, g * GRP : (g + 1) * GRP, :, :]
                else:
                    src_ap = p[:, :]
                    dst_ap = dst[:, g * QSZ : (g + 1) * QSZ]
                if g % 2 == 0:
                    cp = nc.vector.tensor_copy(out=dst_ap, in_=src_ap)
                    dve_copies.append(cp)
                else:
                    nc.scalar.copy(out=dst_ap, in_=src_ap)
                if pass_idx == 2:
                    # store each block as soon as its copy lands; fine
                    # granularity keeps the post-kernel DMA drain short
                    nc.sync.dma_start(
                        out=y_out[:, g * QSZ : (g + 1) * QSZ],
                        in_=v3[:, g * QSZ : (g + 1) * QSZ],
                    )
            return dve_copies

        def pass_boundary(dve_copies):
            # The first matmul of the next pass depends on all copies of
            # the previous pass (true all-to-all).  The DVE-side copy deps
            # are absorbed by a tiny dummy matmul so the first real matmul
            # only carries the ACT-side wait (the LDWEIGHTS instruction
            # encoding holds one sync wait).  The dummy writes quarter 0,
            # which only DVE copies read, so it is covered by its own waits.
            mmi = nc.tensor.matmul(
                out=ps_all[0:32, 0:NB],
                lhsT=braw[:, 0:32],
                rhs=braw[:, 0:NB],
                start=True,
                stop=True,
            )
            for cp in dve_copies:
                add_dep_helper(
                    mmi.ins, cp.ins, sync=True, reason="pass boundary wait split"
                )

        # pass 1: conv D.  V0[d, w*128+h]; chunk w -> [d, h] contiguous;
        #   out [h, d'] for chunk w --split--> V1[h, (d'//8)*1024 + w*8 + d'%8]
        # pass 2: conv H.  split chunk d' -> [h, w]; out [w, h'] for chunk d'
        #   --split--> V2[w, (h'//8)*1024 + d'*8 + h'%8]
        # pass 3: conv W.  split chunk h' -> [w, d']; out [d', w'] for chunk
        #   h' --flat--> V3[d, h*128 + w] == y layout
        d1 = conv_pass(v0, v1, 0, False, True, 0)
        pass_boundary(d1)
        d2 = conv_pass(v1, v2, 1, True, True, 1)
        pass_boundary(d2)
        conv_pass(v2, v3, 2, True, False, 2)

    nc.compile()
    _cache["v2"] = nc
    return nc


def _prep_inputs(x, sigma_x, sigma_y, sigma_z):
    gx = _gauss1d(float(sigma_x), SPACING[0])
    gy = _gauss1d(float(sigma_y), SPACING[1])
    gz = _gauss1d(float(sigma_z), SPACING[2])
    bmats = np.concatenate(
        [_banded(gx, NB), _banded(gy, NB), _banded(gz, NB)], axis=1
    ).astype(np.float16)
    # host-side: cast to fp16 and swap (h, w) so V0 is [d, (w, h)]
    x = np.asarray(x).reshape(N_CORES, P, P, P).astype(np.float16)
    x = np.ascontiguousarray(x.transpose(0, 1, 3, 2)).reshape(N_CORES, P, HW)
    in_maps = [{"x": x[i], "bmats": bmats} for i in range(N_CORES)]
    return in_maps


def _run(x, sigma_x, sigma_y, sigma_z, trace=False):
    from concourse.bass_utils import run_bass_kernel_spmd

    nc = _build()
    in_maps = _prep_inputs(x, sigma_x, sigma_y, sigma_z)
    res = run_bass_kernel_spmd(nc, in_maps, core_ids=list(range(N_CORES)), trace=trace)
    y = np.stack([np.asarray(res.results[i]["y"]) for i in range(N_CORES)])
    y = y.reshape(N_CORES, 1, P, P, P).astype(np.float32)
    return y, res


def kernel(x, sigma_x, sigma_y, sigma_z):
    y, _ = _run(x, sigma_x, sigma_y, sigma_z)
    return y


# revision 37
# speedup vs baseline: 1.0529x; 1.0529x over previous
"""Trainium2 Bass kernel for nn_Blurring_Model: 3D Gaussian blur (9^3 PSF)
on x[8,1,128,128,128] fp32, data-parallel over batch across 8 NeuronCores.

Method (per core, volume V[128,128,128]):
  The 3D PSF is separable: three 9-tap 1D convolutions along D, H, W.
  Each 1D conv along the SBUF partition axis is a matmul with a banded
  128x128 matrix B (B[d, d'] = g[d - d' + 4]); zero band entries handle
  the 'same' zero padding exactly.

  Every pass uses the volume chunk as the matmul's stationary operand
  (lhsT) and streams B, so out = chunk^T @ B both convolves the current
  partition axis AND rotates the next axis onto partitions ("rotation
  matmul") - no explicit transposes.

  v2 design notes (measured on HW, each vs the previous state):
  - fp16 on-chip everywhere (matmuls N=128 stream in 128 PE cycles;
    the f32r path streamed N=256 at half the rate).
  - x is pre-transposed on the HOST to [d, (w, h)] so pass-1 stationary
    chunks are contiguous 128-col blocks; the input DMA is split into
    16 chunk-blocks on one HWDGE ring (the 4MB load is HBM-bound, a
    second ring does not help) and pass 1 chases it.
  - Intermediate volumes use a d'-split layout
        V[p, (n//8)*1024 + m*8 + (n%8)]
    (m = next pass's stationary axis, n = chunk index): weight loads
    read at a single 16B stride and the PSUM evacuation copies keep
    8-element contiguous runs.  Fully-strided 2-byte weight loads
    degraded the copy engines' SBUF throughput 60-90% (sub-bank
    hammering); element-wise scatter copies were ~3.5x slow.  This
    layout keeps the PE at K=8/8 (2.4 GHz, ~56-62ns per LDW+MM pair)
    with the copies pipelined at ~1.1-1.2us per 1024 elems.
  - One PSUM tile spanning all 8 banks as 4 rotating quarters; each
    group's quarter is evacuated by one copy, alternating DVE/ACT.
  - Output volume is fp16 (host upcasts); each 1024-block is DMA'd on
    the Sync queue as soon as its copy lands (short post-kernel drain).
  - ~2us PE warmup covers the first chunk's DMA latency and starts the
    HAM un-throttle without delaying pass-1's first evacuation copies.

  Layouts:
    V0 [d, w*128 + h]        --pass1 conv D (chunk w)--> out [h, d']
    V1 [h, split(d', w)]     --pass2 conv H (chunk d')--> out [w, h']
    V2 [w, split(h', d')]    --pass3 conv W (chunk h')--> out [d', w']
    V3 [d, h*128 + w]        == final y layout, DMA'd out as fp16.
"""

import sys

if "/opt/trn_rl_repo" not in sys.path:
    sys.path.insert(0, "/opt/trn_rl_repo")

import numpy as np

KERNEL_SIZE = 9
SPACING = (4.0, 4.0, 4.0)
CENTER = (KERNEL_SIZE - 1) / 2.0
PAD = (KERNEL_SIZE - 1) // 2
P = 128
HW = P * P
N_CORES = 8

GRP = 8            # matmul chunks per PSUM group / copy
NGRP = P // GRP    # groups per pass
NB = 128           # B matrix columns (= matmul N)
# PE warmup matmuls (~2us): covers the first input chunk's DMA latency and
# starts the HAM un-throttle without delaying pass-1's first evacuation
# copies (pass 1's tail is copy-bound, so a late start costs wall time;
# swept 12/20/28/48 on HW -- 20 gave the best 3-run mean).
N_WARM = 20

_cache = {}


def _gauss1d(sigma, spacing):
    s = float(sigma) / spacing
    xs = np.arange(KERNEL_SIZE, dtype=np.float64)
    g = np.exp(-((xs - CENTER) ** 2) / (2.0 * s * s))
    g = g / g.sum()
    return g.astype(np.float32)


def _banded(g, ncols):
    # B[d, d'] = g[d - d' + PAD] for |d - d'| <= PAD, else 0.
    B = np.zeros((P, ncols), dtype=np.float32)
    d = np.arange(P)
    for i in range(KERNEL_SIZE):
        off = i - PAD
        dp = d - off
        m = (dp >= 0) & (dp < P)
        B[d[m], dp[m]] = g[i]
    return B


def _build():
    """Builds the SPMD Bass module (single program, run on 8 cores)."""
    if "v2" in _cache:
        return _cache["v2"]

    from contextlib import ExitStack

    import concourse.bacc as bacc
    import concourse.mybir as mybir
    import concourse.tile as tile
    from concourse.tile_rust import add_dep_helper

    f32 = mybir.dt.float32
    f16 = mybir.dt.float16

    nc = bacc.Bacc(trn_type="TRN2", target_bir_lowering=False, debug=False)
    x_in = nc.declare_dram_parameter("x", [P, HW], f16, isOutput=False).ap()
    b_in = nc.declare_dram_parameter("bmats", [P, 3 * NB], f16, isOutput=False).ap()
    y_out = nc.declare_dram_parameter("y", [P, HW], f16, isOutput=True).ap()

    with ExitStack() as ctx:
        tc = ctx.enter_context(tile.TileContext(nc))
        vol = ctx.enter_context(tc.tile_pool(name="vol", bufs=4))
        consts = ctx.enter_context(tc.tile_pool(name="consts", bufs=1))
        pspool = ctx.enter_context(tc.tile_pool(name="ps", bufs=1, space="PSUM"))

        braw = consts.tile([P, 3 * NB], f16, name="braw", tag="braw")
        # braw rides the Scalar HWDGE ring (one early descriptor, before any
        # copies are queued there) so the Sync ring starts streaming input
        # chunk 0 immediately at preamble end.
        nc.scalar.dma_start(out=braw[:], in_=b_in[:])
        scratch = consts.tile([P, 128], f32, name="scratch", tag="scratch")

        v0 = vol.tile([P, HW], f16, name="v0", tag="vol")
        v1 = vol.tile([P, HW], f16, name="v1", tag="vol")
        v2 = vol.tile([P, HW], f16, name="v2", tag="vol")
        v3 = vol.tile([P, HW], f16, name="v3", tag="vol")

        # Input DMA, split into 16 chunk-blocks that pass-1 groups chase,
        # all on the Sync HWDGE ring (issued after braw).  The 4MB load is
        # HBM-bandwidth-bound (~358 GB/s per core), so a second ring cannot
        # speed it up -- it only delays the chase chunks.
        for g in range(NGRP):
            nc.sync.dma_start(
                out=v0[:, g * (GRP * NB) : (g + 1) * (GRP * NB)],
                in_=x_in[:, g * (GRP * NB) : (g + 1) * (GRP * NB)],
            )

        # One persistent PSUM tile spanning all 8 banks, divided into 4
        # rotating quarters (2 banks each) indexed by group%4.  Each group's
        # quarter is evacuated by one flat 1024-elem copy, alternating
        # Vector (even groups -> quarters 0/2) and Scalar (odd -> 1/3).
        QSZ = GRP * NB
        ps_all = pspool.tile([P, 4 * QSZ], f32, name="ps_all", tag="ps")

        def ps_q(g):
            q = g % 4
            return ps_all[:, q * QSZ : (q + 1) * QSZ]

        # Warm the ACT tables (Copy) and the PE HAM clock gate while the
        # first input chunk's DMA is in flight.
        nc.scalar.copy(out=scratch[:], in_=braw[:, 0:128])
        for _ in range(N_WARM):
            nc.tensor.matmul(
                out=ps_q(3)[:, 0:NB],
                lhsT=braw[:, 0:128],
                rhs=braw[:, 0:NB],
                start=True,
                stop=True,
            )

        SPL = 16  # d'-split lane width (elements per contiguous run)

        def split_chunk(src, i):
            # d'-split source V[p, nh*SPL*128 + m*SPL + nl] (n = nh*SPL+nl):
            # chunk i -> [p, m:128] single stride SPL (legal 2-D weights AP)
            return src.rearrange("p (nh m nl) -> p nh nl m", nh=P // SPL, m=P, nl=SPL)[
                :, i // SPL, i % SPL, :
            ]

        def block_chunk(src, i):
            return src[:, i * 128 : (i + 1) * 128]

        def conv_pass(src, dst, b_idx, src_split, dst_split, pass_idx):
            # Weight loads with a 2-byte fully-strided AP degrade the copy
            # engines' SBUF throughput by 60-90% (sub-bank hammering), and
            # element-wise scatter copies are ~3.5x slow (one 16B line touch
            # per 2-byte write).  The compromise: intermediate volumes use a
            # d'-split layout  V[p, (n//8)*1024 + m*8 + (n%8)]  where m is
            # the next pass's stationary axis.  Weight loads then read at a
            # single 16B stride (spread across sub-banks) and the evacuation
            # copies keep 8-element contiguous runs (same 16B-line count as
            # a flat copy).
            b_ap = braw[:, b_idx * NB : (b_idx + 1) * NB]
            chunk_fn = split_chunk if src_split else block_chunk
            dve_copies = []
            for g in range(NGRP):
                p = ps_q(g)
                for c in range(GRP):
                    nc.tensor.matmul(
                        out=p[:, c * NB : (c + 1) * NB],
                        lhsT=chunk_fn(src, g * GRP + c),
                        rhs=b_ap,
                        start=True,
                        stop=True,
                    )
                if dst_split:
                    # PSUM [p, (c, nh, nl)] -> dst[p, nh*SPL*128 + (g*8+c)*SPL + nl]
                    # Iterate (nh, c, nl): for fixed nh the dst offsets
                    # (g*8+c)*SPL + nl sweep 64 contiguous elements, so the
                    # write side coalesces into 128B runs (8x fewer SBUF
                    # line touches than (c, nh, nl) order).
                    src_ap = p.rearrange("p (c nh nl) -> p nh c nl", c=GRP, nh=P // SPL)
                    dst_ap = dst.rearrange(
                        "p (nh m nl) -> p nh m nl", nh=P // SPL, m=P, nl=SPL
                    )[:, :, g * GRP : (g + 1) * GRP, :]
                else:
                    src_ap = p[:, :]
                    dst_ap = dst[:, g * QSZ : (g + 1) * QSZ]
                if g % 2 == 0:
                    cp = nc.vector.tensor_copy(out=dst_ap, in_=src_ap)
                    dve_copies.append(cp)
                else:
                    nc.scalar.copy(out=dst_ap, in_=src_ap)
                if pass_idx == 2:
                    # store each block as soon as its copy lands; fine
                    # granularity keeps the post-kernel DMA drain short
                    nc.sync.dma_start(
                        out=y_out[:, g * QSZ : (g + 1) * QSZ],
                        in_=v3[:, g * QSZ : (g + 1) * QSZ],
                    )
            return dve_copies

        def pass_boundary(dve_copies):
            # The first matmul of the next pass depends on all copies of
            # the previous pass (true all-to-all).  The DVE-side copy deps
            # are absorbed by a tiny dummy matmul so the first real matmul
            # only carries the ACT-side wait (the LDWEIGHTS instruction
            # encoding holds one sync wait).  The dummy writes quarter 0,
            # which only DVE copies read, so it is covered by its own waits.
            mmi = nc.tensor.matmul(
                out=ps_all[0:32, 0:NB],
                lhsT=braw[:, 0:32],
                rhs=braw[:, 0:NB],
                start=True,
                stop=True,
            )
            for cp in dve_copies:
                add_dep_helper(
                    mmi.ins, cp.ins, sync=True, reason="pass boundary wait split"
                )

        # pass 1: conv D.  V0[d, w*128+h]; chunk w -> [d, h] contiguous;
        #   out [h, d'] for chunk w --split--> V1[h, (d'//8)*1024 + w*8 + d'%8]
        # pass 2: conv H.  split chunk d' -> [h, w]; out [w, h'] for chunk d'
        #   --split--> V2[w, (h'//8)*1024 + d'*8 + h'%8]
        # pass 3: conv W.  split chunk h' -> [w, d']; out [d', w'] for chunk
        #   h' --flat--> V3[d, h*128 + w] == y layout
        d1 = conv_pass(v0, v1, 0, False, True, 0)
        pass_boundary(d1)
        d2 = conv_pass(v1, v2, 1, True, True, 1)
        pass_boundary(d2)
        conv_pass(v2, v3, 2, True, False, 2)

    nc.compile()
    _cache["v2"] = nc
    return nc


def _prep_inputs(x, sigma_x, sigma_y, sigma_z):
    gx = _gauss1d(float(sigma_x), SPACING[0])
    gy = _gauss1d(float(sigma_y), SPACING[1])
    gz = _gauss1d(float(sigma_z), SPACING[2])
    bmats = np.concatenate(
        [_banded(gx, NB), _banded(gy, NB), _banded(gz, NB)], axis=1
    ).astype(np.float16)
    # host-side: cast to fp16 and swap (h, w) so V0 is [d, (w, h)]
    x = np.asarray(x).reshape(N_CORES, P, P, P).astype(np.float16)
    x = np.ascontiguousarray(x.transpose(0, 1, 3, 2)).reshape(N_CORES, P, HW)
    in_maps = [{"x": x[i], "bmats": bmats} for i in range(N_CORES)]
    return in_maps


def _run(x, sigma_x, sigma_y, sigma_z, trace=False):
    from concourse.bass_utils import run_bass_kernel_spmd

    nc = _build()
    in_maps = _prep_inputs(x, sigma_x, sigma_y, sigma_z)
    res = run_bass_kernel_spmd(nc, in_maps, core_ids=list(range(N_CORES)), trace=trace)
    y = np.stack([np.asarray(res.results[i]["y"]) for i in range(N_CORES)])
    y = y.reshape(N_CORES, 1, P, P, P).astype(np.float32)
    return y, res


def kernel(x, sigma_x, sigma_y, sigma_z):
    y, _ = _run(x, sigma_x, sigma_y, sigma_z)
    return y
